# revision 4
# baseline (speedup 1.0000x reference)
"""Trainium2 Bass kernel for nn_Conv2DSum (logconv1x1_2d / SPN sum layer).

Math: out[b,h,w,s] = logsumexp_c( x[b,h,w,c] + log_softmax(acc)[c,s] )
Since w = softmax(acc) along c sums to 1, the result equals
    out = log( exp(x) @ w )
a convex combination of exp(x_c): numerically safe in fp32/fp16 for
N(0,1)-scale inputs. Tolerance is rel<2e-2; an fp16 end-to-end pipeline
measures ~9e-4, so both HBM directions ship fp16 (halves DMA traffic).

Per core (batch-sharded 8 ways: 4 batches = 65536 rows of 32 ch),
8 tiles of [128, 2048] fp16 in the natural row-major layout
(partition p holds 64 consecutive rows' 32-channel chunks):

  DMA in -> ACT exp (SBUF->SBUF fp16) -> DVE StreamTranspose (32x32
  blocks, SBUF->SBUF) -> PE matmul with a block-diagonal weight as the
  STATIONARY operand (loaded once; moving = transposed exp tile, N=512
  per matmul, fp16 at 1 cycle/row) -> ACT ln (PSUM f32 -> SBUF fp16)
  -> DMA out.

The 32x32 block transpose of the natural layout puts (rowgroup g,
channel c) on partitions; the block-diag weight wblk[(g,c),(g,s)]=w[c,s]
contracts each row's 32 channels against its own diagonal block, so one
[K=128, M=128, N=512] matmul covers 2048 row-outputs. Output rows land
permuted in DRAM; the host inverse-permutes (pure numpy reshape).

No PE transposes, no identity, no PSUM round-trip before the matmul;
PSUM holds only the matmul output (2 x 4 banks, double buffered).
"""

from contextlib import ExitStack

import numpy as np

import concourse.bass as bass
import concourse.tile as tile
from concourse import mybir

# Problem shape (hardcoded per contest rules)
B, H, W, C_IN, N_SUMS = 32, 128, 128, 32, 32
N_CORES = 8
B_PER_CORE = B // N_CORES              # 4
ROWS_PER_CORE = B_PER_CORE * H * W     # 65536
FREE = 2048                            # big-tile free dim (64 rows x 32 ch)
N_TILES = ROWS_PER_CORE * C_IN // (128 * FREE)   # 8

F16 = mybir.dt.float16
F32 = mybir.dt.float32


def build_kernel(nc: bass.Bass):
    x_d = nc.dram_tensor("x", [N_TILES, 128, FREE], F16, kind="ExternalInput").ap()
    wblk_d = nc.dram_tensor("w_blk", [128, 128], F16, kind="ExternalInput").ap()
    out_d = nc.dram_tensor("out", [N_TILES, 128, FREE], F16, kind="ExternalOutput").ap()

    with tile.TileContext(nc) as tc, ExitStack() as ctx:
        const_pool = ctx.enter_context(tc.tile_pool(name="const", bufs=1))
        # all N_TILES input buffers live at once: the single HWDGE queue is
        # loaded with every input DMA up front, so inputs always lead the
        # scalar engine and output DMAs queue behind them
        x_pool = ctx.enter_context(tc.tile_pool(name="x", bufs=N_TILES))
        p_pool = ctx.enter_context(tc.tile_pool(name="p", bufs=2))
        pT_pool = ctx.enter_context(tc.tile_pool(name="pT", bufs=2))
        o_pool = ctx.enter_context(tc.tile_pool(name="o", bufs=4))
        psO_pool = ctx.enter_context(tc.tile_pool(name="psO", bufs=2, space="PSUM"))

        half = FREE // 2

        wblk = const_pool.tile([128, 128], F16, tag="wblk")
        nc.sync.dma_start(wblk[:], wblk_d)
        xts = {}
        for t in range(N_TILES):
            xt = x_pool.tile([128, FREE], F16, tag="xt")
            if t == 0:
                # halves: exp(0) can start on the first half while the
                # second is still in flight
                nc.sync.dma_start(xt[:, 0:half], x_d[t, :, 0:half])
                nc.sync.dma_start(xt[:, half:FREE], x_d[t, :, half:FREE])
            else:
                nc.sync.dma_start(xt[:], x_d[t])
            xts[t] = xt

        # tiny dummy activations up front: the ACT table load (~1.3us)
        # overlaps the first x DMA instead of the critical path
        dummy = const_pool.tile([128, 8], F16, tag="dummy")
        nc.gpsimd.memset(dummy[:], 1.0)
        warm = const_pool.tile([128, 8], F16, tag="warm")
        nc.scalar.activation(warm[:], dummy[:], mybir.ActivationFunctionType.Exp)
        nc.scalar.activation(warm[:], dummy[:], mybir.ActivationFunctionType.Ln)

        # software-pipelined emission: ACT stream is exp(0), exp(1), ln(0),
        # exp(2), ln(1), ... so ln(t-1)'s matmuls are long done when the
        # scalar engine reaches it -- ACT (the bottleneck) never stalls.
        prev = None  # (psO, ot) of tile t-1
        for t in range(N_TILES):
            xt = xts.pop(t)
            pt = p_pool.tile([128, FREE], F16, tag="pt")
            if t == 0:
                nc.scalar.activation(
                    pt[:, 0:half], xt[:, 0:half], mybir.ActivationFunctionType.Exp
                )
                nc.scalar.activation(
                    pt[:, half:FREE],
                    xt[:, half:FREE],
                    mybir.ActivationFunctionType.Exp,
                )
            else:
                nc.scalar.activation(pt[:], xt[:], mybir.ActivationFunctionType.Exp)
            ptT = pT_pool.tile([128, FREE], F16, tag="ptT")
            nc.vector.transpose(ptT[:], pt[:])
            psO = psO_pool.tile([128, FREE], F32)
            for k in range(FREE // 512):
                nc.tensor.matmul(
                    psO[:, bass.ts(k, 512)],
                    wblk[:],
                    ptT[:, bass.ts(k, 512)],
                    start=True,
                    stop=True,
                )
            if prev is not None:
                ps_prev, o_prev, t_prev = prev
                nc.scalar.activation(
                    o_prev[:], ps_prev[:], mybir.ActivationFunctionType.Ln
                )
                nc.sync.dma_start(out_d[t_prev], o_prev[:])
            ot = o_pool.tile([128, FREE], F16, tag="ot")
            prev = (psO, ot, t)
        # last tile: ln + store in halves so the final DMA overlaps the
        # final ln instead of serializing entirely after it
        ps_prev, o_prev, t_prev = prev
        nc.scalar.activation(
            o_prev[:, 0:half], ps_prev[:, 0:half], mybir.ActivationFunctionType.Ln
        )
        nc.sync.dma_start(out_d[t_prev, :, 0:half], o_prev[:, 0:half])
        nc.scalar.activation(
            o_prev[:, half:FREE],
            ps_prev[:, half:FREE],
            mybir.ActivationFunctionType.Ln,
        )
        nc.sync.dma_start(out_d[t_prev, :, half:FREE], o_prev[:, half:FREE])
    return nc


# walrus rejects >1 embedded sync-wait on engine-instruction structs
# (Matmult/Activation/DMA...). The NX sequencer executes embedded waits in
# stream order anyway, so spilling all-but-one wait onto dedicated nops
# immediately before the instruction is semantically identical.
_SPLIT_TYPES = (
    "InstMatmult",
    "InstLdweights",
    "InstActivation",
    "InstDMACopy",
    "InstMemset",
    "InstTensorTensor",
    "InstTensorScalarPtr",
    "InstCopy",
    "InstTensorReduce",
    "InstStreamTranspose",
    "InstDrain",
    "InstNoOp",
)


def _split_embedded_waits(nc: bass.Bass):
    for fn in nc.m.functions:
        for blk in fn.blocks:
            insts = blk.instructions
            out = []
            for inst in insts:
                si = inst.sync_info
                if (
                    si is not None
                    and si.on_wait
                    and len(si.on_wait) > 1
                    and type(inst).__name__ in _SPLIT_TYPES
                ):
                    waits = list(si.on_wait)
                    for i, w in enumerate(waits[:-1]):
                        nop = mybir.InstNoOp(
                            name=f"{inst.name}-sw{i}",
                            engine=inst.engine,
                            sync_info=mybir.SyncInfo(on_wait=[w], on_update=[]),
                            bass_nofuse=True,
                        )
                        out.append(nop)
                    inst.sync_info = mybir.SyncInfo(
                        on_wait=[waits[-1]], on_update=list(si.on_update)
                    )
                out.append(inst)
            if len(out) != len(insts):
                blk.instructions[:] = out


def _host_weights(accumulators: np.ndarray) -> np.ndarray:
    """log_softmax over c of [1,1,Cin,S] accumulators -> exp -> block-diag."""
    acc = np.asarray(accumulators, dtype=np.float64)[0, 0]      # [Cin, S]
    m = acc.max(axis=0, keepdims=True)
    e = np.exp(acc - m)
    w = (e / e.sum(axis=0, keepdims=True)).astype(np.float32)   # [Cin, S]
    w_blk = np.zeros((128, 128), dtype=np.float16)
    for g in range(4):
        w_blk[32 * g : 32 * g + 32, 32 * g : 32 * g + 32] = w.astype(np.float16)
    return w_blk


def _shard_x(x: np.ndarray) -> list[np.ndarray]:
    """Full f32 x [B,H,W,C] -> per-core fp16 [N_TILES, 128, FREE] views."""
    x16 = np.ascontiguousarray(x.reshape(-1, C_IN)).astype(np.float16)
    x16 = x16.reshape(N_CORES, N_TILES, 128, FREE)
    return [x16[c] for c in range(N_CORES)]


def _unshard_out(outs: list[np.ndarray]) -> np.ndarray:
    """Per-core fp16 [N_TILES, 128, FREE] -> full f32 [B,H,W,C] output.

    Kernel output layout: out[t, 32g+s, 128j+32q+p] = res[row, s] with
    row = 2048g + 64p + 4j + q (within tile t's 8192 rows).
    """
    full = np.empty((B, H, W, N_SUMS), dtype=np.float32)
    fr = full.reshape(N_CORES, ROWS_PER_CORE, N_SUMS)
    for c, o in enumerate(outs):
        arr = o.reshape(N_TILES, 4, 32, 16, 4, 32)      # [t, g, s, j, q, p]
        fr[c] = (
            arr.transpose(0, 1, 5, 3, 4, 2)             # [t, g, p, j, q, s]
            .reshape(ROWS_PER_CORE, N_SUMS)
            .astype(np.float32)
        )
    return full


_CACHE: dict = {}


def make_bass():
    return bass.Bass("TRN2", debug=False, num_swdge_queues=4)


def kernel(**inputs: np.ndarray) -> np.ndarray:
    from concourse.bass_utils import run_bass_kernel_spmd

    x = np.asarray(inputs["x"], dtype=np.float32)
    acc = np.asarray(inputs["accumulators"], dtype=np.float32)

    w_blk = _host_weights(acc)

    if "nc" not in _CACHE:
        nc = build_kernel(make_bass())
        # HW path only: CoreSim can't digest post-hoc inserted nops
        _split_embedded_waits(nc)
        _CACHE["nc"] = nc
    nc = _CACHE["nc"]

    shards = _shard_x(x)
    in_maps = [{"x": shards[c], "w_blk": w_blk} for c in range(N_CORES)]

    res = run_bass_kernel_spmd(nc, in_maps, core_ids=list(range(N_CORES)))
    outs = [np.asarray(res.results[c]["out"]) for c in range(N_CORES)]
    return _unshard_out(outs)


# revision 5
# speedup vs baseline: 1.0878x; 1.0878x over previous
"""Trainium2 Bass kernel for nn_Conv2DSum (logconv1x1_2d / SPN sum layer).

Math: out[b,h,w,s] = logsumexp_c( x[b,h,w,c] + log_softmax(acc)[c,s] )
Since w = softmax(acc) along c sums to 1, the result equals
    out = log( exp(x) @ w )
a convex combination of exp(x_c): numerically safe in fp32/fp16 for
N(0,1)-scale inputs. Tolerance is rel<2e-2; an fp16 end-to-end pipeline
measures ~9e-4, so both HBM directions ship fp16 (halves DMA traffic).

Per core (batch-sharded 8 ways: 4 batches = 65536 rows of 32 ch),
8 tiles of [128, 2048] fp16 in the natural row-major layout
(partition p holds 64 consecutive rows' 32-channel chunks):

  DMA in -> ACT exp (SBUF->SBUF fp16) -> DVE StreamTranspose (32x32
  blocks, SBUF->SBUF) -> PE matmul with a block-diagonal weight as the
  STATIONARY operand (loaded once; moving = transposed exp tile, N=512
  per matmul, fp16 at 1 cycle/row) -> ACT ln (PSUM f32 -> SBUF fp16)
  -> DMA out.

The 32x32 block transpose of the natural layout puts (rowgroup g,
channel c) on partitions; the block-diag weight wblk[(g,c),(g,s)]=w[c,s]
contracts each row's 32 channels against its own diagonal block, so one
[K=128, M=128, N=512] matmul covers 2048 row-outputs. Output rows land
permuted in DRAM; the host inverse-permutes (pure numpy reshape).

No PE transposes, no identity, no PSUM round-trip before the matmul;
PSUM holds only the matmul output (2 x 4 banks, double buffered).
"""

from contextlib import ExitStack

import numpy as np

import concourse.bass as bass
import concourse.tile as tile
from concourse import mybir

# Problem shape (hardcoded per contest rules)
B, H, W, C_IN, N_SUMS = 32, 128, 128, 32, 32
N_CORES = 8
B_PER_CORE = B // N_CORES              # 4
ROWS_PER_CORE = B_PER_CORE * H * W     # 65536
FREE = 2048                            # big-tile free dim (64 rows x 32 ch)
N_TILES = ROWS_PER_CORE * C_IN // (128 * FREE)   # 8

F16 = mybir.dt.float16
F32 = mybir.dt.float32


def build_kernel(nc: bass.Bass):
    x_d = nc.dram_tensor("x", [N_TILES, 128, FREE], F16, kind="ExternalInput").ap()
    wblk_d = nc.dram_tensor("w_blk", [128, 128], F16, kind="ExternalInput").ap()
    out_d = nc.dram_tensor("out", [N_TILES, 128, FREE], F16, kind="ExternalOutput").ap()

    with tile.TileContext(nc) as tc, ExitStack() as ctx:
        const_pool = ctx.enter_context(tc.tile_pool(name="const", bufs=1))
        # all N_TILES input buffers live at once: the single HWDGE queue is
        # loaded with every input DMA up front, so inputs always lead the
        # scalar engine and output DMAs queue behind them
        x_pool = ctx.enter_context(tc.tile_pool(name="x", bufs=N_TILES))
        p_pool = ctx.enter_context(tc.tile_pool(name="p", bufs=2))
        pT_pool = ctx.enter_context(tc.tile_pool(name="pT", bufs=2))
        o_pool = ctx.enter_context(tc.tile_pool(name="o", bufs=4))
        psO_pool = ctx.enter_context(tc.tile_pool(name="psO", bufs=2, space="PSUM"))

        half = FREE // 2

        wblk = const_pool.tile([128, 128], F16, tag="wblk")
        nc.sync.dma_start(wblk[:], wblk_d)
        # first chunk small so exp(0) starts as early as possible
        X0_CHUNKS = (512, 512, 1024)
        xts = {}
        for t in range(N_TILES):
            xt = x_pool.tile([128, FREE], F16, tag="xt")
            if t == 0:
                off = 0
                for c in X0_CHUNKS:
                    nc.sync.dma_start(
                        xt[:, off : off + c], x_d[t, :, off : off + c]
                    )
                    off += c
            else:
                nc.sync.dma_start(xt[:], x_d[t])
            xts[t] = xt

        # tiny dummy activations up front: the ACT table load (~1.3us)
        # overlaps the first x DMA instead of the critical path. memset on
        # DVE (not gpsimd) to keep the Pool/Q7 engine out of the program.
        dummy = const_pool.tile([128, 8], F16, tag="dummy")
        nc.vector.memset(dummy[:], 1.0)
        warm = const_pool.tile([128, 8], F16, tag="warm")
        nc.scalar.activation(warm[:], dummy[:], mybir.ActivationFunctionType.Exp)
        nc.scalar.activation(warm[:], dummy[:], mybir.ActivationFunctionType.Ln)

        # software-pipelined emission with a 2-tile ln lag: ACT stream is
        # exp0, exp1, exp2, ln0, exp3, ln1, ..., exp7, ln5, ln6, ln7 -- the
        # last tile's transpose/matmul chain hides under ln5/ln6 instead of
        # serializing after the final exp.
        pending = []  # [(psO, ot, t)] not yet ln'd
        for t in range(N_TILES):
            xt = xts.pop(t)
            pt = p_pool.tile([128, FREE], F16, tag="pt")
            if t == 0:
                off = 0
                for c in X0_CHUNKS:
                    nc.scalar.activation(
                        pt[:, off : off + c],
                        xt[:, off : off + c],
                        mybir.ActivationFunctionType.Exp,
                    )
                    off += c
            else:
                nc.scalar.activation(pt[:], xt[:], mybir.ActivationFunctionType.Exp)
            if len(pending) == 2:
                ps_prev, o_prev, t_prev = pending.pop(0)
                nc.scalar.activation(
                    o_prev[:], ps_prev[:], mybir.ActivationFunctionType.Ln
                )
                nc.sync.dma_start(out_d[t_prev], o_prev[:])
            ptT = pT_pool.tile([128, FREE], F16, tag="ptT")
            nc.vector.transpose(ptT[:], pt[:])
            psO = psO_pool.tile([128, FREE], F32)
            for k in range(FREE // 512):
                nc.tensor.matmul(
                    psO[:, bass.ts(k, 512)],
                    wblk[:],
                    ptT[:, bass.ts(k, 512)],
                    start=True,
                    stop=True,
                )
            ot = o_pool.tile([128, FREE], F16, tag="ot")
            pending.append((psO, ot, t))
        # drain: ln6 whole, then ln7 in halves so the final DMA overlaps
        # the final ln instead of serializing entirely after it
        ps_prev, o_prev, t_prev = pending.pop(0)
        nc.scalar.activation(o_prev[:], ps_prev[:], mybir.ActivationFunctionType.Ln)
        nc.sync.dma_start(out_d[t_prev], o_prev[:])
        ps_prev, o_prev, t_prev = pending.pop(0)
        nc.scalar.activation(
            o_prev[:, 0:half], ps_prev[:, 0:half], mybir.ActivationFunctionType.Ln
        )
        nc.sync.dma_start(out_d[t_prev, :, 0:half], o_prev[:, 0:half])
        nc.scalar.activation(
            o_prev[:, half:FREE],
            ps_prev[:, half:FREE],
            mybir.ActivationFunctionType.Ln,
        )
        nc.sync.dma_start(out_d[t_prev, :, half:FREE], o_prev[:, half:FREE])
    return nc


# walrus rejects >1 embedded sync-wait on engine-instruction structs
# (Matmult/Activation/DMA...). The NX sequencer executes embedded waits in
# stream order anyway, so spilling all-but-one wait onto dedicated nops
# immediately before the instruction is semantically identical.
_SPLIT_TYPES = (
    "InstMatmult",
    "InstLdweights",
    "InstActivation",
    "InstDMACopy",
    "InstMemset",
    "InstTensorTensor",
    "InstTensorScalarPtr",
    "InstCopy",
    "InstTensorReduce",
    "InstStreamTranspose",
    "InstDrain",
    "InstNoOp",
)


def _split_embedded_waits(nc: bass.Bass):
    for fn in nc.m.functions:
        for blk in fn.blocks:
            insts = blk.instructions
            out = []
            for inst in insts:
                si = inst.sync_info
                if (
                    si is not None
                    and si.on_wait
                    and len(si.on_wait) > 1
                    and type(inst).__name__ in _SPLIT_TYPES
                ):
                    waits = list(si.on_wait)
                    for i, w in enumerate(waits[:-1]):
                        nop = mybir.InstNoOp(
                            name=f"{inst.name}-sw{i}",
                            engine=inst.engine,
                            sync_info=mybir.SyncInfo(on_wait=[w], on_update=[]),
                            bass_nofuse=True,
                        )
                        out.append(nop)
                    inst.sync_info = mybir.SyncInfo(
                        on_wait=[waits[-1]], on_update=list(si.on_update)
                    )
                out.append(inst)
            if len(out) != len(insts):
                blk.instructions[:] = out


def _host_weights(accumulators: np.ndarray) -> np.ndarray:
    """log_softmax over c of [1,1,Cin,S] accumulators -> exp -> block-diag."""
    acc = np.asarray(accumulators, dtype=np.float64)[0, 0]      # [Cin, S]
    m = acc.max(axis=0, keepdims=True)
    e = np.exp(acc - m)
    w = (e / e.sum(axis=0, keepdims=True)).astype(np.float32)   # [Cin, S]
    w_blk = np.zeros((128, 128), dtype=np.float16)
    for g in range(4):
        w_blk[32 * g : 32 * g + 32, 32 * g : 32 * g + 32] = w.astype(np.float16)
    return w_blk


def _shard_x(x: np.ndarray) -> list[np.ndarray]:
    """Full f32 x [B,H,W,C] -> per-core fp16 [N_TILES, 128, FREE] views."""
    x16 = np.ascontiguousarray(x.reshape(-1, C_IN)).astype(np.float16)
    x16 = x16.reshape(N_CORES, N_TILES, 128, FREE)
    return [x16[c] for c in range(N_CORES)]


def _unshard_out(outs: list[np.ndarray]) -> np.ndarray:
    """Per-core fp16 [N_TILES, 128, FREE] -> full f32 [B,H,W,C] output.

    Kernel output layout: out[t, 32g+s, 128j+32q+p] = res[row, s] with
    row = 2048g + 64p + 4j + q (within tile t's 8192 rows).
    """
    full = np.empty((B, H, W, N_SUMS), dtype=np.float32)
    fr = full.reshape(N_CORES, ROWS_PER_CORE, N_SUMS)
    for c, o in enumerate(outs):
        arr = o.reshape(N_TILES, 4, 32, 16, 4, 32)      # [t, g, s, j, q, p]
        fr[c] = (
            arr.transpose(0, 1, 5, 3, 4, 2)             # [t, g, p, j, q, s]
            .reshape(ROWS_PER_CORE, N_SUMS)
            .astype(np.float32)
        )
    return full


_CACHE: dict = {}


def make_bass():
    return bass.Bass("TRN2", debug=False, num_swdge_queues=4)


def kernel(**inputs: np.ndarray) -> np.ndarray:
    from concourse.bass_utils import run_bass_kernel_spmd

    x = np.asarray(inputs["x"], dtype=np.float32)
    acc = np.asarray(inputs["accumulators"], dtype=np.float32)

    w_blk = _host_weights(acc)

    if "nc" not in _CACHE:
        nc = build_kernel(make_bass())
        # HW path only: CoreSim can't digest post-hoc inserted nops
        _split_embedded_waits(nc)
        _CACHE["nc"] = nc
    nc = _CACHE["nc"]

    shards = _shard_x(x)
    in_maps = [{"x": shards[c], "w_blk": w_blk} for c in range(N_CORES)]

    res = run_bass_kernel_spmd(nc, in_maps, core_ids=list(range(N_CORES)))
    outs = [np.asarray(res.results[c]["out"]) for c in range(N_CORES)]
    return _unshard_out(outs)
